# revision 22
# baseline (speedup 1.0000x reference)
"""Balanced dice loss (histogram binning) on 8 Trainium2 NeuronCores.

Math: with t ∈ {0,1} and p = sigmoid(x), the loss needs four global sums:
    S_t   = Σ t            (the bincount)
    S_pt  = Σ p·t
    S_pp  = Σ p²
    S_ppt = Σ p²·t
Then with c1 = S_t, c0 = N − c1, w0 = 1/(c0+s)², w1 = 1/(c1+s)²:
    intersection = w1·S_pt
    denominator  = w0·(S_pp − S_ppt) + w1·(S_ppt + c1)
    dice = 1 − (2·I + s)/(D + s)

Device kernel (data-parallel over 8 cores, batch-sharded). Host casts
x to bf16 and target to int16 (values are {0,1}): HBM traffic drops from
32MB to 16MB per core. Per [128,4096] tile:
    ACT : p = sigmoid(x) (bf16) + for tiles 1..7 a copy pass over the
          int16 t with accum_out → S_t partials, delayed one tile on the
          ACT queue so sigmoid(i+1) isn't stuck behind copy(i)
    DVE : u = p·t (bf16×int16, 2x mode), w = u·p, sq = p·p (all 2x)
    PE  : ones[128,128]-stationary column-sum chains over 512-col chunks:
          Σu → S_pt, Σw → S_ppt, Σsq → S_pp. A tile's 24 chain matmuls
          are emitted one tile late so they run as one uninterrupted
          burst (the tensor engine p-state ramps to full clock only
          after ~3µs of continuous execution). Tile 0's S_t goes through
          a PE chain over the int16 t bitcast to fp16 (value 1 =
          denormal 2^-24, summed exactly in f32 PSUM) so the PE has
          DVE-independent work at stream start.
Each chain alternates two PSUM banks and accumulates across all tiles.
The ACT S_t partials [P, NT] are partition-reduced on device by one tiny
f32 matmul; everything leaves through a single [1, 8·512+NT] output row.
Host sums in float64 and finishes the scalar math.
"""

import numpy as np
import ml_dtypes

import concourse.bacc as bacc
import concourse.mybir as mybir
from concourse.bass_utils import run_bass_kernel_spmd
from concourse.tile import TileContext

N_CORES = 8
P = 128
TOTAL = 32 * 1024 * 1024  # elements in the full problem
PER_CORE = TOTAL // N_CORES  # 4,194,304
FREE = PER_CORE // P  # 32,768 per partition
F = 4096  # tile free-dim
NT = FREE // F  # 8 tiles per core
MMN = 512  # matmul moving free-dim (one PSUM bank; ISA max)
SMOOTH = 1e-05

PS_NAMES = ("ua", "ub", "wa", "wb", "qa", "qb", "ta", "tb")
PE_T_TILES = (0, 7)  # tiles whose S_t goes through the PE denorm chain

_nc_cache = None


def _slices(i):
    # sub-instruction column slices within a tile
    if i == 0:
        return [(0, 512), (512, 2048), (2048, 4096)]
    if i == NT - 1:
        return [(0, 1024), (1024, 2048), (2048, 3072), (3072, 4096)]
    return [(0, F)]


def _build_bass():
    nc = bacc.Bacc(None, target_bir_lowering=False)
    x = nc.dram_tensor("input", [P, FREE], mybir.dt.bfloat16, kind="ExternalInput")
    t = nc.dram_tensor("target", [P, FREE], mybir.dt.int16, kind="ExternalInput")
    # 8 chain partials ([1, MMN] each, order PS_NAMES) then NT S_t partials
    o_fin = nc.dram_tensor(
        "o_fin", [1, 8 * MMN + NT], mybir.dt.float32, kind="ExternalOutput"
    )
    o_st = nc.dram_tensor("o_st", [P, NT], mybir.dt.float32, kind="ExternalOutput")

    with TileContext(nc) as tc:
        with (
            tc.tile_pool(name="work", bufs=2) as pool,
            tc.tile_pool(name="stats", bufs=1) as spool,
            tc.tile_pool(name="ps", bufs=1, space="PSUM") as psum,
        ):
            ones = spool.tile([P, P], mybir.dt.bfloat16, tag="ones")
            nc.any.memset(ones, 1.0)
            s_t_acc = spool.tile([P, NT], mybir.dt.float32, tag="s_t_acc")
            nc.vector.memset(s_t_acc, 0.0)

            ps_all = psum.tile(
                [P, 8, MMN], mybir.dt.float32, tag="ps_all", name="ps_all"
            )
            ps = {name: ps_all[:, k, :] for k, name in enumerate(PS_NAMES)}
            ps_first = {name: True for name in PS_NAMES}

            # emit all loads first so the sync queue streams them; t before
            # x within each tile (the PE t-chain and DVE need t first)
            xts, tts = [], []
            for i in range(NT):
                xt = pool.tile([P, F], mybir.dt.bfloat16, tag="xt", bufs=4)
                tt = pool.tile([P, F], mybir.dt.int16, tag="tt", bufs=4)
                eng = nc.gpsimd if i == 0 else nc.sync
                for a, b in _slices(i):
                    eng.dma_start(tt[:, a:b], t[:, i * F + a : i * F + b])
                    eng.dma_start(xt[:, a:b], x[:, i * F + a : i * F + b])
                xts.append(xt)
                tts.append(tt)

            # chunks per chain across the whole kernel (for stop flags)
            totals = {"u": FREE // MMN, "w": FREE // MMN, "q": FREE // MMN,
                      "t": len(PE_T_TILES) * F // MMN}
            emitted = {k: 0 for k in totals}

            def chain(stream, pair, moving):
                emitted[stream] += 1
                name = pair[emitted[stream] % 2]  # alternate banks
                nc.tensor.matmul(
                    ps[name],
                    ones[:],
                    moving,
                    start=ps_first[name],
                    stop=emitted[stream] > totals[stream] - 2,
                )
                ps_first[name] = False

            def emit_chains(i, a, b):
                uwq = uwqs[i]
                for j in range(a // MMN, b // MMN):
                    chain("u", ("ua", "ub"), uwq[:, j * MMN : (j + 1) * MMN])
                for j in range(a // MMN, b // MMN):
                    chain(
                        "w", ("wa", "wb"),
                        uwq[:, F + j * MMN : F + (j + 1) * MMN],
                    )
                for j in range(a // MMN, b // MMN):
                    chain(
                        "q", ("qa", "qb"),
                        uwq[:, 2 * F + j * MMN : 2 * F + (j + 1) * MMN],
                    )

            t_scr = spool.tile([P, F], mybir.dt.bfloat16, tag="t_scr")
            fin = spool.tile([1, 8 * MMN], mybir.dt.float32, tag="fin")
            uwqs = []

            for i in range(NT):
                xt, tt = xts[i], tts[i]
                p_ = pool.tile([P, F], mybir.dt.bfloat16, tag="p", bufs=4)
                uwq = pool.tile([P, 3 * F], mybir.dt.bfloat16, tag="uwq", bufs=3)
                uwqs.append(uwq)
                t16 = tt[:].bitcast(mybir.dt.float16)

                if 0 < i:
                    # chains for the PREVIOUS tile: its u/w/sq are complete,
                    # so the 24 matmuls run as one continuous burst  [PE]
                    emit_chains(i - 1, 0, F)
                for a, b in _slices(i):
                    u = uwq[:, a:b]
                    w = uwq[:, F + a : F + b]
                    sq = uwq[:, 2 * F + a : 2 * F + b]
                    pv = p_[:, a:b]

                    if i in PE_T_TILES:
                        # S_t chain first: only needs the t DMA  [PE]
                        for j in range(a // MMN, b // MMN):
                            chain("t", ("ta", "tb"), t16[:, j * MMN : (j + 1) * MMN])

                    # p = sigmoid(x)  [ACT]
                    nc.scalar.activation(
                        pv, xt[:, a:b], mybir.ActivationFunctionType.Sigmoid
                    )
                    # products (all 2-byte operands -> DVE 2x mode)  [DVE]
                    nc.vector.tensor_tensor(
                        out=u, in0=pv, in1=tt[:, a:b], op=mybir.AluOpType.mult
                    )
                    nc.vector.tensor_tensor(
                        out=w, in0=u, in1=pv, op=mybir.AluOpType.mult
                    )
                    nc.vector.tensor_tensor(
                        out=sq, in0=pv, in1=pv, op=mybir.AluOpType.mult
                    )
                    if i == NT - 1:
                        # last tile: chains right away, per slice (drain)
                        emit_chains(i, a, b)

                # S_t via copy-with-accum for the PREVIOUS tile (delayed
                # one tile so sigmoid(i+1) isn't stuck behind copy(i) on
                # the ACT queue); output is a dead scratch  [ACT]
                for j in (i - 1, i) if i == NT - 1 else (i - 1,):
                    if 0 <= j and j not in PE_T_TILES:
                        with tc.high_priority(offset=-30):
                            nc.scalar.activation(
                                t_scr[:],
                                tts[j][:],
                                mybir.ActivationFunctionType.Copy,
                                accum_out=s_t_acc[:, j : j + 1],
                            )
                        if j == max(k for k in range(NT) if k not in PE_T_TILES):
                            # s_t_acc complete; ship it mid-drain so its DMA
                            # latency hides under the remaining chain work
                            nc.sync.dma_start(o_st[:], s_t_acc[:])

            # evict the psum partial rows; split between DVE & ACT
            for k, name in enumerate(PS_NAMES):
                dst = fin[:, k * MMN : (k + 1) * MMN]
                if k % 2 == 0:
                    nc.vector.tensor_copy(dst, ps[name][0:1, :])
                else:
                    nc.scalar.copy(dst, ps[name][0:1, :])
            nc.sync.dma_start(o_fin[:, : 8 * MMN], fin[:, : 8 * MMN])
    nc.finalize()
    return nc


def _get_nc():
    global _nc_cache
    if _nc_cache is None:
        _nc_cache = _build_bass()
    return _nc_cache


def kernel(input, target, _trace=False):
    x = (
        np.ascontiguousarray(np.asarray(input, dtype=np.float32))
        .reshape(N_CORES, P, FREE)
        .astype(ml_dtypes.bfloat16)
    )
    t = (
        np.ascontiguousarray(np.asarray(target))
        .reshape(N_CORES, P, FREE)
        .astype(np.int16)
    )
    in_maps = [{"input": x[i], "target": t[i]} for i in range(N_CORES)]

    nc = _get_nc()
    res = run_bass_kernel_spmd(
        nc, in_maps, core_ids=list(range(N_CORES)), trace=_trace
    )
    kernel.last_results = res

    s_pt = s_ppt = s_pp = s_t = 0.0
    for r in res.results:
        f = r["o_fin"].astype(np.float64)[0]
        s_pt += float(f[0 : 2 * MMN].sum())
        s_ppt += float(f[2 * MMN : 4 * MMN].sum())
        s_pp += float(f[4 * MMN : 6 * MMN].sum())
        s_t += float(f[6 * MMN : 8 * MMN].sum()) * (2.0**24)
        s_t += float(r["o_st"].astype(np.float64).sum())

    c1 = float(s_t)
    c0 = float(TOTAL - s_t)
    w0 = 1.0 / (c0 + SMOOTH) ** 2
    w1 = 1.0 / (c1 + SMOOTH) ** 2
    intersection = w1 * s_pt
    denominator = w0 * (s_pp - s_ppt) + w1 * (s_ppt + c1)
    dice = 1.0 - (2.0 * intersection + SMOOTH) / (denominator + SMOOTH)
    return np.asarray(dice, dtype=np.float32)


# revision 27
# speedup vs baseline: 1.1390x; 1.1390x over previous
"""Balanced dice loss (histogram binning) on 8 Trainium2 NeuronCores.

Math: with t ∈ {0,1} and p = sigmoid(x), the loss needs four global sums:
    S_t   = Σ t            (the bincount)
    S_pt  = Σ p·t
    S_pp  = Σ p²
    S_ppt = Σ p²·t
Then with c1 = S_t, c0 = N − c1, w0 = 1/(c0+s)², w1 = 1/(c1+s)²:
    intersection = w1·S_pt
    denominator  = w0·(S_pp − S_ppt) + w1·(S_ppt + c1)
    dice = 1 − (2·I + s)/(D + s)

Device kernel (data-parallel over 8 cores, batch-sharded). Host casts
x to bf16 and target to int16 (values are {0,1}): HBM traffic drops from
32MB to 16MB per core, and every device operand is 2 bytes. Per
[128,4096] tile:
    ACT : p = sigmoid(x) → bf16 (the only full ACT pass), plus for tiles
          {3,4,5,6} a copy pass over the int16 t with accum_out → S_t
          partials. The copies are emitted one tile late so sigmoid(i+1)
          isn't queued behind copy(i).
    DVE : u = p·t (bf16×int16), w = u·p, sq = p·p — all in the DVE's
          2x parallel mode (every operand 2-byte, packed).
    PE  : ones[128,128]-stationary column-sum chains over 512-col
          chunks: Σu → S_pt, Σw → S_ppt, Σsq → S_pp. A tile's 24 chain
          matmuls are emitted one tile late so they run as one
          uninterrupted burst. Tiles {0,1,2,7} sum their S_t on the PE
          instead: a chain over the int16 t bitcast to fp16 (value 1 =
          denormal 2^-24, summed exactly in f32 PSUM); tiles 1,2 are
          emitted ahead of the delayed chains, filling the PE's
          pipeline-fill dependency gaps with DMA-only work.
All four chains alternate two PSUM banks of one spanning 8-bank PSUM
tile and accumulate across all tiles; one eviction + a split output DMA
at the end. The ACT S_t partials ship mid-drain. The first tile is
emitted in 512/1536/2048-col slices (compute starts as soon as 128KB
lands), the last in 1024-col slices (short drain). Host sums in float64
and finishes the scalar math.

Perf notes (measured): all-bf16/int16 keeps DVE at 2x (~0.56ns/col);
the PE runs ~1.2-1.4GHz when all 8 cores are active (the 2.4GHz p-state
is only reachable single-core); HBM streams ~350-400GB/s/core during
the load phase. The three compute engines each carry ~50-57µs of work
under a ~46µs DMA stream; exec ≈ 80µs ≈ fill + max-engine + drain.
"""

import numpy as np
import ml_dtypes

import concourse.bacc as bacc
import concourse.mybir as mybir
from concourse.bass_utils import run_bass_kernel_spmd
from concourse.tile import TileContext

N_CORES = 8
P = 128
TOTAL = 32 * 1024 * 1024  # elements in the full problem
PER_CORE = TOTAL // N_CORES  # 4,194,304
FREE = PER_CORE // P  # 32,768 per partition
F = 4096  # tile free-dim
NT = FREE // F  # 8 tiles per core
MMN = 512  # matmul moving free-dim (one PSUM bank; ISA max)
SMOOTH = 1e-05

PS_NAMES = ("ua", "ub", "wa", "wb", "qa", "qb", "ta", "tb")
PE_T_TILES = (0, 1, 2, 7)  # tiles whose S_t goes through the PE denorm chain

_nc_cache = None


def _slices(i):
    # sub-instruction column slices within a tile
    if i == 0:
        return [(0, 512), (512, 2048), (2048, 4096)]
    if i == NT - 1:
        return [(0, 1024), (1024, 2048), (2048, 3072), (3072, 4096)]
    return [(0, F)]


def _build_bass():
    nc = bacc.Bacc(None, target_bir_lowering=False)
    x = nc.dram_tensor("input", [P, FREE], mybir.dt.bfloat16, kind="ExternalInput")
    t = nc.dram_tensor("target", [P, FREE], mybir.dt.int16, kind="ExternalInput")
    # 8 chain partials ([1, MMN] each, order PS_NAMES) then NT S_t partials
    o_fin = nc.dram_tensor(
        "o_fin", [1, 8 * MMN + NT], mybir.dt.float32, kind="ExternalOutput"
    )
    o_st = nc.dram_tensor("o_st", [P, NT], mybir.dt.float32, kind="ExternalOutput")

    with TileContext(nc) as tc:
        with (
            tc.tile_pool(name="work", bufs=2) as pool,
            tc.tile_pool(name="stats", bufs=1) as spool,
            tc.tile_pool(name="ps", bufs=1, space="PSUM") as psum,
        ):
            ones = spool.tile([P, P], mybir.dt.bfloat16, tag="ones")
            nc.any.memset(ones, 1.0)
            s_t_acc = spool.tile([P, NT], mybir.dt.float32, tag="s_t_acc")
            nc.vector.memset(s_t_acc, 0.0)

            ps_all = psum.tile(
                [P, 8, MMN], mybir.dt.float32, tag="ps_all", name="ps_all"
            )
            ps = {name: ps_all[:, k, :] for k, name in enumerate(PS_NAMES)}
            ps_first = {name: True for name in PS_NAMES}

            # emit all loads first so the sync queue streams them; t before
            # x within each tile (the PE t-chain and DVE need t first)
            xts, tts = [], []
            for i in range(NT):
                xt = pool.tile([P, F], mybir.dt.bfloat16, tag="xt", bufs=4)
                tt = pool.tile([P, F], mybir.dt.int16, tag="tt", bufs=4)
                for a, b in _slices(i):
                    nc.sync.dma_start(tt[:, a:b], t[:, i * F + a : i * F + b])
                    nc.sync.dma_start(xt[:, a:b], x[:, i * F + a : i * F + b])
                xts.append(xt)
                tts.append(tt)

            # chunks per chain across the whole kernel (for stop flags)
            totals = {"u": FREE // MMN, "w": FREE // MMN, "q": FREE // MMN,
                      "t": len(PE_T_TILES) * F // MMN}
            emitted = {k: 0 for k in totals}

            def chain(stream, pair, moving):
                emitted[stream] += 1
                name = pair[emitted[stream] % 2]  # alternate banks
                nc.tensor.matmul(
                    ps[name],
                    ones[:],
                    moving,
                    start=ps_first[name],
                    stop=emitted[stream] > totals[stream] - 2,
                )
                ps_first[name] = False

            def emit_chains(i, a, b):
                uwq = uwqs[i]
                for j in range(a // MMN, b // MMN):
                    chain("u", ("ua", "ub"), uwq[:, j * MMN : (j + 1) * MMN])
                for j in range(a // MMN, b // MMN):
                    chain(
                        "w", ("wa", "wb"),
                        uwq[:, F + j * MMN : F + (j + 1) * MMN],
                    )
                for j in range(a // MMN, b // MMN):
                    chain(
                        "q", ("qa", "qb"),
                        uwq[:, 2 * F + j * MMN : 2 * F + (j + 1) * MMN],
                    )

            t_scr = spool.tile([P, F], mybir.dt.bfloat16, tag="t_scr")
            fin = spool.tile([1, 8 * MMN], mybir.dt.float32, tag="fin")
            uwqs = []

            for i in range(NT):
                xt, tt = xts[i], tts[i]
                p_ = pool.tile([P, F], mybir.dt.bfloat16, tag="p", bufs=4)
                uwq = pool.tile([P, 3 * F], mybir.dt.bfloat16, tag="uwq", bufs=3)
                uwqs.append(uwq)
                t16 = tt[:].bitcast(mybir.dt.float16)

                if i in PE_T_TILES and i != NT - 1 and i > 0:
                    # S_t chunks only need the t DMA; in front of the
                    # previous tile's chains they fill PE dependency gaps
                    for j in range(F // MMN):
                        chain("t", ("ta", "tb"), t16[:, j * MMN : (j + 1) * MMN])
                if 0 < i:
                    # chains for the PREVIOUS tile: its u/w/sq are complete,
                    # so the 24 matmuls run as one continuous burst  [PE]
                    emit_chains(i - 1, 0, F)
                for a, b in _slices(i):
                    u = uwq[:, a:b]
                    w = uwq[:, F + a : F + b]
                    sq = uwq[:, 2 * F + a : 2 * F + b]
                    pv = p_[:, a:b]

                    if i in PE_T_TILES and (i == 0 or i == NT - 1):
                        # S_t chain first: only needs the t DMA  [PE]
                        for j in range(a // MMN, b // MMN):
                            chain("t", ("ta", "tb"), t16[:, j * MMN : (j + 1) * MMN])

                    # p = sigmoid(x)  [ACT]
                    nc.scalar.activation(
                        pv, xt[:, a:b], mybir.ActivationFunctionType.Sigmoid
                    )
                    # products (all 2-byte operands -> DVE 2x mode)  [DVE]
                    nc.vector.tensor_tensor(
                        out=u, in0=pv, in1=tt[:, a:b], op=mybir.AluOpType.mult
                    )
                    nc.vector.tensor_tensor(
                        out=w, in0=u, in1=pv, op=mybir.AluOpType.mult
                    )
                    nc.vector.tensor_tensor(
                        out=sq, in0=pv, in1=pv, op=mybir.AluOpType.mult
                    )
                    if i == NT - 1:
                        # last tile: chains right away, per slice (drain)
                        emit_chains(i, a, b)

                # S_t via copy-with-accum for the PREVIOUS tile (delayed
                # one tile so sigmoid(i+1) isn't stuck behind copy(i) on
                # the ACT queue); output is a dead scratch  [ACT]
                for j in (i - 1, i) if i == NT - 1 else (i - 1,):
                    if 0 <= j and j not in PE_T_TILES:
                        with tc.high_priority(offset=-30):
                            nc.scalar.activation(
                                t_scr[:],
                                tts[j][:],
                                mybir.ActivationFunctionType.Copy,
                                accum_out=s_t_acc[:, j : j + 1],
                            )
                        if j == max(k for k in range(NT) if k not in PE_T_TILES):
                            # s_t_acc complete; ship it mid-drain so its DMA
                            # latency hides under the remaining chain work
                            nc.sync.dma_start(o_st[:], s_t_acc[:])

            # evict the psum partial rows; split between DVE & ACT, with
            # the output DMA split so its latency overlaps the eviction
            for k, name in enumerate(PS_NAMES):
                dst = fin[:, k * MMN : (k + 1) * MMN]
                if k % 2 == 0:
                    nc.vector.tensor_copy(dst, ps[name][0:1, :])
                else:
                    nc.scalar.copy(dst, ps[name][0:1, :])
                if k == 3:
                    nc.sync.dma_start(o_fin[:, : 4 * MMN], fin[:, : 4 * MMN])
            nc.sync.dma_start(o_fin[:, 4 * MMN : 8 * MMN], fin[:, 4 * MMN :])
    nc.finalize()
    return nc


def _get_nc():
    global _nc_cache
    if _nc_cache is None:
        _nc_cache = _build_bass()
    return _nc_cache


def kernel(input, target, _trace=False):
    x = (
        np.ascontiguousarray(np.asarray(input, dtype=np.float32))
        .reshape(N_CORES, P, FREE)
        .astype(ml_dtypes.bfloat16)
    )
    t = (
        np.ascontiguousarray(np.asarray(target))
        .reshape(N_CORES, P, FREE)
        .astype(np.int16)
    )
    in_maps = [{"input": x[i], "target": t[i]} for i in range(N_CORES)]

    nc = _get_nc()
    res = run_bass_kernel_spmd(
        nc, in_maps, core_ids=list(range(N_CORES)), trace=_trace
    )
    kernel.last_results = res

    s_pt = s_ppt = s_pp = s_t = 0.0
    for r in res.results:
        f = r["o_fin"].astype(np.float64)[0]
        s_pt += float(f[0 : 2 * MMN].sum())
        s_ppt += float(f[2 * MMN : 4 * MMN].sum())
        s_pp += float(f[4 * MMN : 6 * MMN].sum())
        s_t += float(f[6 * MMN : 8 * MMN].sum()) * (2.0**24)
        s_t += float(r["o_st"].astype(np.float64).sum())

    c1 = float(s_t)
    c0 = float(TOTAL - s_t)
    w0 = 1.0 / (c0 + SMOOTH) ** 2
    w1 = 1.0 / (c1 + SMOOTH) ** 2
    intersection = w1 * s_pt
    denominator = w0 * (s_pp - s_ppt) + w1 * (s_ppt + c1)
    dice = 1.0 - (2.0 * intersection + SMOOTH) / (denominator + SMOOTH)
    return np.asarray(dice, dtype=np.float32)
